# revision 1
# baseline (speedup 1.0000x reference)
"""Trainium2 Bass kernel for MemoryEfficientFlashAttention (B=2,S=2048,HID=2048,H=16,HKV=8,D=128,CHUNK=512).

Sharding: 8 cores = 2 batches x 4 head-groups (4 q heads / 2 kv heads per core).
Each core computes q/k/v projections (+RoPE), the chunked flash-attention
recurrence, and a row-sharded partial of the output projection (transposed).
Host sums the 4 partials per batch and adds bo.

Math: the reference's scan step is algebraically
    o_j = (o_{j-1} * e^{m_{j-1}} + Y_j) / (e^{m_{j-1}} + S_j)
with Y_j = exp(sc_j) @ V_j, S_j = rowsum exp(sc_j), m_j = running max.
Unrolled:  o_n = sum_j Y_j * C_{j-1} / (C_n * e^{m_n}),  C_j = prod_{l<=j} d_l,
    d_l = e^{m_{l-1}-m_l} + T_l,  T_l = rowsum exp(sc_l - m_l).
Pass 1 computes the (m, T, d, lnC) chains per row; pass 2 recomputes scores
transposed and accumulates  u = sum_j exp(sc_j^T + w_j - gamma) @ V  directly
in PSUM, with w_j = lnC_{j-1} and gamma = m_n + lnC_n (+ ln d_n if the
globally-last kv chunk was processed, reproducing the reference's final o/d
divide).  u is then exactly the final attention output; exponents are <= 0 so
everything is numerically stable.
"""

import os
import sys
from contextlib import ExitStack

import numpy as np

sys.path.insert(0, "/opt/trn_rl_repo")
os.environ.setdefault("MYCRO_LOCAL_CACHE", "1")

import concourse.bass as bass  # noqa: E402
import concourse.tile as tile  # noqa: E402
from concourse import bacc, mybir  # noqa: E402
from concourse.bass_utils import run_bass_kernel_spmd  # noqa: E402

B, S, HID = 2, 2048, 2048
H, HKV, D = 16, 8, 128
CHUNK = 512
THETA = 1000000.0
NCORES = 8
HL = H // (NCORES // B)      # 4 local q heads
KVL = HKV // (NCORES // B)   # 2 local kv heads
NQ = S // CHUNK              # 4 chunks
NT = HID // 128              # 16 hid tiles
SCALE = 1.0 / np.sqrt(np.float32(D))

F32 = mybir.dt.float32
F32R = mybir.dt.float32r
BF16 = mybir.dt.bfloat16
Alu = mybir.AluOpType
Act = mybir.ActivationFunctionType

# 'bf16pair' = exact-enough two-term bf16 inject; 'f32r' = single fast inject
INJECT_MODE = os.environ.get("FA_INJECT_MODE", "bf16pair")

_CACHE = {}


def _f32r_round(a):
    """Round fp32 to the fp32r format (1s/8e/11m in the high 20 bits):
    round-to-nearest-even at mantissa bit 12."""
    u = np.ascontiguousarray(a, dtype=np.float32).view(np.uint32).copy()
    low = u & np.uint32(0xFFF)
    base = u & ~np.uint32(0xFFF)
    lsb = (base >> 12) & np.uint32(1)
    round_up = (low > 0x800) | ((low == 0x800) & (lsb == 1))
    out = base + (round_up.astype(np.uint32) << 12)
    return out.view(np.float32)


def _rope_tables():
    inv_freq = 1.0 / (THETA ** (np.arange(0, D, 2, dtype=np.float32) / D))
    pos = np.arange(S, dtype=np.float32)
    freqs = pos[:, None].astype(np.float32) * inv_freq[None, :]
    emb = np.concatenate([freqs, freqs], axis=-1)  # [S, D]
    cosT = np.cos(emb).astype(np.float32).T.copy()
    sinT = np.sin(emb).astype(np.float32).T.copy()
    return cosT, sinT  # [D, S]


def _classify_mask(attention_mask):
    """Per (qi, j) CHUNKxCHUNK block: 'zero' | 'neg' | 'mixed', merged across
    batches so the SPMD program is identical on all cores."""
    kinds = {}
    for qi in range(NQ):
        for j in range(NQ):
            kind = "neg"
            for b in range(B):
                blk = attention_mask[b, 0, qi * CHUNK:(qi + 1) * CHUNK,
                                     j * CHUNK:(j + 1) * CHUNK]
                if np.all(blk == 0.0):
                    k = "zero"
                elif np.all(blk <= -1e6):
                    k = "neg"
                else:
                    k = "mixed"
                if k == "mixed" or kind == "mixed":
                    kind = "mixed"
                elif k == "zero" or kind == "zero":
                    kind = "zero"
            kinds[(qi, j)] = kind
    plan = {}
    for qi in range(NQ):
        processed = []
        for j in range(NQ):
            k = kinds[(qi, j)]
            if k == "neg" and len(processed) > 0:
                continue  # identity step under the reference's fp32 exp underflow
            processed.append((j, k != "zero"))
        plan[qi] = processed
    mask_blocks = sorted({(qi, j) for qi in range(NQ)
                          for (j, need) in plan[qi] if need})
    return plan, mask_blocks


def _mm(nc, out, lhsT, rhs, start, stop):
    nc.tensor.matmul(out, lhsT, rhs, start=start, stop=stop)


def _emit(tc, ap, plan, mix_idx):
    nc = tc.nc

    with ExitStack() as top:
        # ---------------- persistent tensors ----------------
        pers = top.enter_context(tc.tile_pool(name="pers", bufs=1))
        QT = pers.tile([128, HL, S], F32R)             # rope'd q^T  [d, h, s]
        KT = pers.tile([128, KVL, S], F32R)            # rope'd k^T  [d, kv, s]
        V = pers.tile([128, S // 128, KVL * D], F32R)  # v natural [s_p, s_t, kv*d]
        I128 = pers.tile([128, 128], F32R)
        nc.sync.dma_start(I128, ap["imat"])
        I128f = pers.tile([128, 128], F32)
        nc.sync.dma_start(I128f, ap["imat"].bitcast(F32))
        ones1 = pers.tile([1, 128], F32R)
        nc.sync.dma_start(ones1, ap["ones1"])
        ones1b = pers.tile([1, 128], BF16)
        nc.vector.memset(ones1b, 1.0)
        R128 = pers.tile([128, 128], F32R)
        nc.sync.dma_start(R128, ap["rmat"])
        bqk = pers.tile([128, HL + KVL], F32)
        nc.sync.dma_start(bqk, ap["bqk"])
        bv = pers.tile([1, KVL * D], F32R)
        nc.sync.dma_start(bv, ap["bv"])

        # ---------------- phase 1: projections + rope ----------------
        with ExitStack() as ph1:
            xt_pool = ph1.enter_context(tc.tile_pool(name="xt", bufs=2))
            w_pool = ph1.enter_context(tc.tile_pool(name="wcol", bufs=2))
            wv_pool = ph1.enter_context(tc.tile_pool(name="wvp", bufs=1))
            cs_pool = ph1.enter_context(tc.tile_pool(name="cs", bufs=2))
            raw_pool = ph1.enter_context(tc.tile_pool(name="raw", bufs=2))
            t_pool = ph1.enter_context(tc.tile_pool(name="ropetmp", bufs=2))
            psP = ph1.enter_context(tc.tile_pool(name="psP", bufs=2, space="PSUM"))
            psR = ph1.enter_context(tc.tile_pool(name="psR", bufs=2, space="PSUM"))
            psV = ph1.enter_context(tc.tile_pool(name="psV", bufs=2, space="PSUM"))

            wv_sb = wv_pool.tile([128, NT, KVL * D], F32R)
            nc.sync.dma_start(wv_sb, ap["wv"].rearrange("(t p) m -> p t m", p=128))

            hsT_r = ap["hsT"].rearrange("(t p) s -> p t s", p=128)
            wqk_r = ap["wqk"].rearrange("(t p) m -> p t m", p=128)

            for sq in range(S // CHUNK):
                ssl = slice(sq * CHUNK, (sq + 1) * CHUNK)
                xt = xt_pool.tile([128, NT, CHUNK], F32R)
                nc.sync.dma_start(xt, hsT_r[:, :, ssl])
                cost = cs_pool.tile([128, CHUNK], F32, tag="cos")
                nc.sync.dma_start(cost, ap["cosT"][:, ssl])
                sint = cs_pool.tile([128, CHUNK], F32, tag="sin")
                nc.sync.dma_start(sint, ap["sinT"][:, ssl])

                # q^T and k^T projections, rope'd
                for m in range(HL + KVL):
                    w = w_pool.tile([128, NT, 128], F32R)
                    nc.sync.dma_start(w, wqk_r[:, :, m * 128:(m + 1) * 128])
                    ps = psP.tile([128, CHUNK], F32)
                    for t in range(NT):
                        _mm(nc, ps, w[:, t], xt[:, t],
                            start=(t == 0), stop=(t == NT - 1))
                    raw = raw_pool.tile([128, CHUNK], F32R)
                    nc.vector.tensor_scalar_add(raw, ps, bqk[:, m:m + 1])
                    pr = psR.tile([128, CHUNK], F32)
                    _mm(nc, pr, R128, raw, start=True, stop=True)
                    t1 = t_pool.tile([128, CHUNK], F32, tag="t1")
                    nc.gpsimd.tensor_mul(t1, raw.bitcast(F32), cost)
                    t2 = t_pool.tile([128, CHUNK], F32, tag="t2")
                    nc.vector.tensor_mul(t2, pr, sint)
                    dest = QT[:, m, ssl] if m < HL else KT[:, m - HL, ssl]
                    nc.vector.tensor_add(dest, t1, t2)

                # v projection (natural layout), bias via K=1 matmul
                for ss in range(CHUNK // 128):
                    pv = psV.tile([128, KVL * D], F32)
                    for t in range(NT):
                        _mm(nc, pv, xt[:, t, ss * 128:(ss + 1) * 128], wv_sb[:, t],
                            start=(t == 0), stop=False)
                    _mm(nc, pv, ones1, bv, start=False, stop=True)
                    nc.vector.tensor_copy(V[:, sq * 4 + ss, :], pv)

        # ---------------- phase 2: attention ----------------
        with ExitStack() as ph2:
            mkN_pool = ph2.enter_context(tc.tile_pool(name="mkN", bufs=2))
            mkT_pool = ph2.enter_context(tc.tile_pool(name="mkT", bufs=1))
            sc_ps = ph2.enter_context(tc.tile_pool(name="scps", bufs=4, space="PSUM"))
            s2_ps = ph2.enter_context(tc.tile_pool(name="s2ps", bufs=2, space="PSUM"))
            u_ps = ph2.enter_context(tc.tile_pool(name="ups", bufs=1, space="PSUM"))
            ch_pool = ph2.enter_context(tc.tile_pool(name="chain", bufs=2))
            ws_pool = ph2.enter_context(tc.tile_pool(name="wstar", bufs=2))
            scr_pool = ph2.enter_context(tc.tile_pool(name="scratch", bufs=3))
            wf_pool = ph2.enter_context(tc.tile_pool(name="wflat", bufs=1))
            p2_pool = ph2.enter_context(tc.tile_pool(name="pprime", bufs=3))
            o2_pool = ph2.enter_context(tc.tile_pool(name="uout", bufs=2))
            wo_pool = ph2.enter_context(tc.tile_pool(name="wo", bufs=3))
            o_pool = ph2.enter_context(tc.tile_pool(name="osb", bufs=3))
            psO = ph2.enter_context(tc.tile_pool(name="psO", bufs=1, space="PSUM"))
            wo_r = ap["wo"].rearrange("(t p) m -> p t m", p=128)

            for qi in range(NQ):
                chunks = plan[qi]  # list of (j, needs_mask)
                nj = len(chunks)
                qsl = slice(qi * CHUNK, (qi + 1) * CHUNK)

                nm = [ch_pool.tile([128, HL * 4], F32, tag=f"nm{p}", name=f"nm{p}")
                      for p in range(2)]
                nc.vector.memset(nm[0], 1e30)
                Tj = ch_pool.tile([128, HL * 4], F32, tag="Tj")
                negmx = ch_pool.tile([128, HL * 4], F32, tag="negmx")
                dm = ch_pool.tile([128, HL * 4], F32, tag="dm")
                pj = ch_pool.tile([128, HL * 4], F32, tag="pj")
                dstore = ws_pool.tile([128, nj, HL * 4], F32, tag="dstore")
                lnq = ws_pool.tile([128, nj, HL * 4], F32, tag="lnq")
                Wadj = ws_pool.tile([128, nj, HL * 4], F32, tag="wadj")

                # ---- pass 1: running max + exp-sum chains ----
                for t, (j, need_mask) in enumerate(chunks):
                    ksl = slice(j * CHUNK, (j + 1) * CHUNK)
                    nmo, nmn = nm[t % 2], nm[(t + 1) % 2]
                    mn = None
                    if need_mask:
                        mn = mkN_pool.tile([128, 4, CHUNK], F32R)
                        nc.sync.dma_start(mn, ap["maskN"][mix_idx[(qi, j)]])
                    for h in range(HL):
                        hc = slice(h * 4, h * 4 + 4)
                        ps_subs = []
                        for sub in range(4):
                            col = h * 4 + sub
                            q0 = qi * CHUNK + sub * 128
                            ps = sc_ps.tile([128, CHUNK], F32)
                            _mm(nc, ps, QT[:, h, q0:q0 + 128], KT[:, h // 2, ksl],
                                start=True, stop=not need_mask)
                            if need_mask:
                                _mm(nc, ps, I128, mn[:, sub, :],
                                    start=False, stop=True)
                            nc.vector.tensor_reduce(
                                negmx[:, col:col + 1], ps,
                                axis=mybir.AxisListType.X, op=Alu.max, negate=True)
                            ps_subs.append(ps)
                        nc.vector.tensor_tensor(nmn[:, hc], nmo[:, hc],
                                                negmx[:, hc], Alu.min)
                        for sub in range(4):
                            col = h * 4 + sub
                            scr2 = scr_pool.tile([128, CHUNK], BF16, tag="exp_out")
                            nc.scalar.activation(
                                scr2, ps_subs[sub], Act.Exp,
                                bias=nmn[:, col:col + 1], scale=1.0,
                                accum_out=Tj[:, col:col + 1])
                    nc.vector.tensor_sub(dm, nmn, nmo)   # = m_old - m_new
                    nc.scalar.activation(pj, dm, Act.Exp)
                    nc.vector.tensor_add(dstore[:, t, :], pj, Tj)

                nm_fin = nm[nj % 2]
                # inject_t = -m_n - ln(prod_{l>=t} d_l * d_n^flag): backward
                # products then ONE batched Ln (avoids Exp<->Ln table thrash)
                if any(j == NQ - 1 for (j, _) in chunks):
                    nc.vector.tensor_mul(dstore[:, nj - 1, :],
                                         dstore[:, nj - 1, :],
                                         dstore[:, nj - 1, :])
                for t in range(nj - 2, -1, -1):
                    nc.vector.tensor_mul(dstore[:, t, :], dstore[:, t, :],
                                         dstore[:, t + 1, :])
                nc.scalar.activation(lnq, dstore, Act.Ln)
                for t in range(nj):
                    nc.vector.tensor_sub(Wadj[:, t, :], nm_fin, lnq[:, t, :])

                # transpose Wadj -> wt2 [nj*HL, 512] (row = (t, h), col = sq)
                wtp = sc_ps.tile([nj * HL, 4, 128], F32, tag="ps", name="wtp")
                wadj_r = Wadj.rearrange("p n (x a) -> p n x a", a=4)
                for sub in range(4):
                    nc.tensor.transpose(wtp[:, sub, :], wadj_r[:, :, :, sub], I128f)
                wt2 = scr_pool.tile([nj * HL, CHUNK], F32, tag="wt2")
                nc.vector.tensor_copy(wt2, wtp)
                # flatten rows onto partition 0 (matmul rhs needs base partition 0)
                if INJECT_MODE == "bf16pair":
                    wt2hi = scr_pool.tile([nj * HL, CHUNK], BF16, tag="wt2hi")
                    nc.vector.tensor_copy(wt2hi, wt2)
                    wt2lo = scr_pool.tile([nj * HL, CHUNK], BF16, tag="wt2lo")
                    nc.vector.tensor_sub(wt2lo, wt2, wt2hi)
                    wthi_f = wf_pool.tile([1, nj * HL, CHUNK], BF16, tag="wthi_f")
                    nc.sync.dma_start(wthi_f, wt2hi)
                    wtlo_f = wf_pool.tile([1, nj * HL, CHUNK], BF16, tag="wtlo_f")
                    nc.sync.dma_start(wtlo_f, wt2lo)
                else:
                    wt2r = scr_pool.tile([nj * HL, CHUNK], F32R, tag="wt2r")
                    nc.vector.tensor_copy(wt2r, wt2)
                    wt_f = wf_pool.tile([1, nj * HL, CHUNK], F32R, tag="wt_f")
                    nc.sync.dma_start(wt_f, wt2r)

                # ---- pass 2: transposed scores + exp + PV accumulate ----
                mtload = {}
                for t, (j, need_mask) in enumerate(chunks):
                    if need_mask:
                        mt = mkT_pool.tile([128, 4, CHUNK], F32R, tag=f"mt{j}")
                        nc.sync.dma_start(mt, ap["maskT"][mix_idx[(qi, j)]])
                        mtload[j] = mt

                ubs = []
                for h in range(HL):
                    up = u_ps.tile([128, CHUNK], F32)
                    for t, (j, need_mask) in enumerate(chunks):
                        for kc in range(4):
                            k0 = j * CHUNK + kc * 128
                            sp = s2_ps.tile([128, CHUNK], F32)
                            _mm(nc, sp, KT[:, h // 2, k0:k0 + 128], QT[:, h, qsl],
                                start=True, stop=False)
                            if need_mask:
                                _mm(nc, sp, I128, mtload[j][:, kc, :],
                                    start=False, stop=False)
                            row = t * HL + h
                            if INJECT_MODE == "bf16pair":
                                nc.tensor.matmul(sp, ones1b, wthi_f[:, row, :],
                                                 start=False, stop=False)
                                nc.tensor.matmul(sp, ones1b, wtlo_f[:, row, :],
                                                 start=False, stop=True)
                            else:
                                _mm(nc, sp, ones1, wt_f[:, row, :],
                                    start=False, stop=True)
                            pp = p2_pool.tile([128, CHUNK], F32R)
                            nc.scalar.activation(pp, sp, Act.Exp)
                            _mm(nc, up, V[:, j * 4 + kc, (h // 2) * D:(h // 2 + 1) * D],
                                pp, start=(t == 0 and kc == 0),
                                stop=(t == nj - 1 and kc == 3))
                    ub = o2_pool.tile([128, CHUNK], F32R, tag=f"ub{h}",
                                      name=f"ub{h}")
                    nc.vector.tensor_copy(ub, up)
                    ubs.append(ub)

                # output projection for this s-chunk (st == qi)
                for mo in range(HID // 128):
                    wo_t = wo_pool.tile([128, HL, 128], F32R)
                    nc.sync.dma_start(wo_t, wo_r[:, :, mo * 128:(mo + 1) * 128])
                    po = psO.tile([128, CHUNK], F32)
                    for t in range(HL):
                        _mm(nc, po, wo_t[:, t], ubs[t],
                            start=(t == 0), stop=(t == HL - 1))
                    ob = o_pool.tile([128, CHUNK], F32)
                    if mo % 2 == 0:
                        nc.scalar.copy(ob, po)
                    else:
                        nc.vector.tensor_copy(ob, po)
                    nc.sync.dma_start(
                        ap["outT"][mo * 128:(mo + 1) * 128, qsl], ob)

def _build_program(plan, mask_blocks):
    nc = bacc.Bacc("TRN2", target_bir_lowering=False, debug=False,
                   enable_asserts=False, num_devices=NCORES)
    ap = {}
    ap["hsT"] = nc.dram_tensor("hsT", [HID, S], F32R, kind="ExternalInput").ap()
    ap["wqk"] = nc.dram_tensor("wqk", [HID, (HL + KVL) * D], F32R, kind="ExternalInput").ap()
    ap["wv"] = nc.dram_tensor("wv", [HID, KVL * D], F32R, kind="ExternalInput").ap()
    ap["wo"] = nc.dram_tensor("wo", [HL * D, HID], F32R, kind="ExternalInput").ap()
    ap["bqk"] = nc.dram_tensor("bqk", [D, HL + KVL], F32, kind="ExternalInput").ap()
    ap["bv"] = nc.dram_tensor("bv", [1, KVL * D], F32R, kind="ExternalInput").ap()
    ap["cosT"] = nc.dram_tensor("cosT", [D, S], F32, kind="ExternalInput").ap()
    ap["sinT"] = nc.dram_tensor("sinT", [D, S], F32, kind="ExternalInput").ap()
    ap["rmat"] = nc.dram_tensor("rmat", [D, D], F32R, kind="ExternalInput").ap()
    ap["imat"] = nc.dram_tensor("imat", [128, 128], F32R, kind="ExternalInput").ap()
    ap["ones1"] = nc.dram_tensor("ones1", [1, 128], F32R, kind="ExternalInput").ap()
    nmix = max(1, len(mask_blocks))
    ap["maskN"] = nc.dram_tensor("maskN", [nmix, 128, 4, CHUNK], F32R, kind="ExternalInput").ap()
    ap["maskT"] = nc.dram_tensor("maskT", [nmix, 128, 4, CHUNK], F32R, kind="ExternalInput").ap()
    ap["outT"] = nc.dram_tensor("outT", [HID, S], F32, kind="ExternalOutput").ap()
    mix_idx = {qj: i for i, qj in enumerate(mask_blocks)}

    with tile.TileContext(nc) as tc:
        _emit(tc, ap, plan, mix_idx)
    nc.compile()
    return nc


def _host_inputs(inputs, mask_blocks):
    hs = np.asarray(inputs["hidden_states"], dtype=np.float32)
    am = np.asarray(inputs["attention_mask"], dtype=np.float32)
    Wq = np.asarray(inputs["Wq"], dtype=np.float32)
    bq = np.asarray(inputs["bq"], dtype=np.float32)
    Wk = np.asarray(inputs["Wk"], dtype=np.float32)
    bk = np.asarray(inputs["bk"], dtype=np.float32)
    Wv = np.asarray(inputs["Wv"], dtype=np.float32)
    bv_ = np.asarray(inputs["bv"], dtype=np.float32)
    Wo = np.asarray(inputs["Wo"], dtype=np.float32)

    cosT, sinT = _rope_tables()
    R = np.zeros((D, D), dtype=np.float32)
    R[64 + np.arange(64), np.arange(64)] = -1.0   # out[d'<64] = -q[d'+64]
    R[np.arange(64), 64 + np.arange(64)] = 1.0    # out[d'>=64] = q[d'-64]
    I = np.eye(128, dtype=np.float32)

    Wq4 = (Wq * SCALE).reshape(HID, H, D)
    bq4 = (bq * SCALE).reshape(H, D)
    Wk4 = Wk.reshape(HID, HKV, D)
    bk4 = bk.reshape(HKV, D)
    Wv4 = Wv.reshape(HID, HKV, D)
    bv4 = bv_.reshape(HKV, D)
    Wo4 = Wo.reshape(H, D, HID)

    nmix = max(1, len(mask_blocks))
    in_maps = []
    for c in range(NCORES):
        b, hg = divmod(c, NCORES // B)
        qh = slice(hg * HL, (hg + 1) * HL)
        kvh = slice(hg * KVL, (hg + 1) * KVL)
        wqk = np.concatenate([
            Wq4[:, qh].reshape(HID, HL * D),
            Wk4[:, kvh].reshape(HID, KVL * D)], axis=1)
        bqk = np.concatenate([bq4[qh], bk4[kvh]], axis=0).T  # [D, HL+KVL]
        mN = np.zeros((nmix, 128, 4, CHUNK), dtype=np.float32)
        mT = np.zeros((nmix, 128, 4, CHUNK), dtype=np.float32)
        for i, (qi, j) in enumerate(mask_blocks):
            blk = am[b, 0, qi * CHUNK:(qi + 1) * CHUNK, j * CHUNK:(j + 1) * CHUNK]
            mN[i] = blk.reshape(4, 128, CHUNK).transpose(1, 0, 2)
            mT[i] = blk.T.reshape(4, 128, CHUNK).transpose(1, 0, 2)
        in_maps.append({
            "hsT": _f32r_round(hs[b].T),
            "wqk": _f32r_round(wqk),
            "wv": _f32r_round(Wv4[:, kvh].reshape(HID, KVL * D)),
            "wo": _f32r_round(Wo4[qh].reshape(HL * D, HID)),
            "bqk": np.ascontiguousarray(bqk),
            "bv": _f32r_round(bv4[kvh].reshape(1, KVL * D)),
            "cosT": cosT,
            "sinT": sinT,
            "rmat": R,
            "imat": I,
            "ones1": np.ones((1, 128), dtype=np.float32),
            "maskN": _f32r_round(mN),
            "maskT": _f32r_round(mT),
        })
    return in_maps


def get_program(inputs):
    am = np.asarray(inputs["attention_mask"], dtype=np.float32)
    plan, mask_blocks = _classify_mask(am)
    key = (str(plan), str(mask_blocks), INJECT_MODE)
    if key not in _CACHE:
        _CACHE[key] = _build_program(plan, mask_blocks)
    return _CACHE[key], plan, mask_blocks


def run(inputs, **spmd_kwargs):
    nc, plan, mask_blocks = get_program(inputs)
    in_maps = _host_inputs(inputs, mask_blocks)
    res = run_bass_kernel_spmd(nc, in_maps, core_ids=list(range(NCORES)),
                               **spmd_kwargs)
    bo = np.asarray(inputs["bo"], dtype=np.float32)
    out = np.empty((B, S, HID), dtype=np.float32)
    gpb = NCORES // B
    for b in range(B):
        acc = np.zeros((HID, S), dtype=np.float32)
        for c in range(b * gpb, (b + 1) * gpb):
            acc += res.results[c]["outT"]
        out[b] = acc.T + bo
    return out, res


def kernel(**inputs) -> np.ndarray:
    out, _ = run(inputs)
    return out



# revision 46
# speedup vs baseline: 1.4743x; 1.4743x over previous
"""Trainium2 Bass kernel for MemoryEfficientFlashAttention (B=2,S=2048,HID=2048,H=16,HKV=8,D=128,CHUNK=512).

Sharding: 8 cores = 2 batches x 4 head-groups (4 q heads / 2 kv heads per core).
Each core computes q/k/v projections (+RoPE), the chunked flash-attention
recurrence, and a row-sharded partial of the output projection (transposed).
Host sums the 4 partials per batch and adds bo.

Math (v2, raw-exp domain): the reference's scan output unrolls to
    out = sum_j exp(sc_j) V_j * Q_j,
    Q_j = Pprod_j / Hprod_j,
    Pprod_j = prod_{l=j..n} E_l,       E_l = e^{running max after block l}
    Hprod_j = extra * prod_{l=j..n} g_l,
    g_1 = s_1,  g_l = E_{l-1} + s_l,   s_l = rowsum exp(sc_l)  (raw)
    extra = g_n if the globally-last kv chunk was processed else E_n.
Scores are small (|sc| <~ 15), so raw exps never overflow fp32 and the
chain needs no logs until the single batched Ln that produces the
per-(block,head) additive inject w_j = ln Q_j for pass 2.

Pass 1 computes raw-exp sums + maxes in [q, k] orientation (Act accum +
cheap bf16 rowmax on DVE); pass 2 recomputes scores transposed, injects
w_j via a K=1 f32r matmul, exps, and accumulates PV directly in PSUM.
Projections/attention use f32r moving operands where N>=256 (full rate)
and bf16 where narrower (KT, masks), keeping precision well inside the
gate while all matmuls run at 1 cycle/row.
"""

import os
import sys
from contextlib import ExitStack

import numpy as np
import ml_dtypes

sys.path.insert(0, "/opt/trn_rl_repo")
os.environ.setdefault("MYCRO_LOCAL_CACHE", "1")

import concourse.bass as bass  # noqa: E402
import concourse.tile as tile  # noqa: E402
from concourse import bacc, mybir  # noqa: E402
from concourse.bass_utils import run_bass_kernel_spmd  # noqa: E402

B, S, HID = 2, 2048, 2048
H, HKV, D = 16, 8, 128
CHUNK = 512
THETA = 1000000.0
NCORES = 8
HL = H // (NCORES // B)      # 4 local q heads
KVL = HKV // (NCORES // B)   # 2 local kv heads
NQ = S // CHUNK              # 4 chunks
NT = HID // 128              # 16 hid tiles
SCALE = 1.0 / np.sqrt(np.float32(D))

F32 = mybir.dt.float32
F32R = mybir.dt.float32r
BF16 = mybir.dt.bfloat16
Alu = mybir.AluOpType
Act = mybir.ActivationFunctionType
BFL = ml_dtypes.bfloat16

_CACHE = {}


def _f32r_round(a):
    """Round fp32 to the fp32r format (1s/8e/11m in the high 20 bits):
    round-to-nearest-even at mantissa bit 12."""
    u = np.ascontiguousarray(a, dtype=np.float32).view(np.uint32).copy()
    low = u & np.uint32(0xFFF)
    base = u & ~np.uint32(0xFFF)
    lsb = (base >> 12) & np.uint32(1)
    round_up = (low > 0x800) | ((low == 0x800) & (lsb == 1))
    out = base + (round_up.astype(np.uint32) << 12)
    return out.view(np.float32)


def _bf16(a):
    return np.ascontiguousarray(np.asarray(a, dtype=np.float32)).astype(BFL)


def _rope_tables():
    inv_freq = 1.0 / (THETA ** (np.arange(0, D, 2, dtype=np.float32) / D))
    pos = np.arange(S, dtype=np.float32)
    freqs = pos[:, None].astype(np.float32) * inv_freq[None, :]
    emb = np.concatenate([freqs, freqs], axis=-1)  # [S, D]
    cosT = np.cos(emb).astype(np.float32).T.copy()
    sinT = np.sin(emb).astype(np.float32).T.copy()
    return cosT, sinT  # [D, S]


def _classify_mask(attention_mask):
    """Per (qi, j) CHUNKxCHUNK block: 'zero' | 'neg' | 'tri' | 'mixed',
    merged across batches so the SPMD program is identical on all cores."""
    tril = np.tril(np.ones((CHUNK, CHUNK), dtype=bool))
    kinds = {}
    for qi in range(NQ):
        for j in range(NQ):
            kind = None
            for b in range(B):
                blk = attention_mask[b, 0, qi * CHUNK:(qi + 1) * CHUNK,
                                     j * CHUNK:(j + 1) * CHUNK]
                if np.all(blk == 0.0):
                    k = "zero"
                elif np.all(blk <= -1e6):
                    k = "neg"
                elif np.all((blk == 0.0) == tril) and np.all(blk[~tril] <= -1e6):
                    k = "tri"
                else:
                    k = "mixed"
                if kind is None or kind == k:
                    kind = k
                else:
                    kind = "mixed"
            kinds[(qi, j)] = kind
    plan = {}
    for qi in range(NQ):
        processed = []
        for j in range(NQ):
            k = kinds[(qi, j)]
            if k == "neg" and len(processed) > 0:
                continue  # identity step under the reference's fp32 exp underflow
            if k == "neg":
                k = "mixed"  # first block kept even if fully masked
            processed.append((j, "full" if k == "zero" else k))
        plan[qi] = processed
    mask_blocks = sorted({(qi, j) for qi in range(NQ)
                          for (j, kind) in plan[qi] if kind == "mixed"})
    return plan, mask_blocks


def _mm(nc, out, lhsT, rhs, start, stop, skip=False):
    nc.tensor.matmul(out, lhsT, rhs, start=start, stop=stop,
                     skip_group_check=skip)


def _emit(tc, ap, plan, mix_idx):
    nc = tc.nc

    with ExitStack() as top:
        # ---------------- persistent tensors + cross-phase pools ----------
        pers = top.enter_context(tc.tile_pool(name="pers", bufs=1))
        QT = pers.tile([128, HL, S], BF16)             # rope'd q^T  [d, h, s]
        KT = pers.tile([128, KVL, S], BF16)            # rope'd k^T  [d, kv, s]
        V = pers.tile([128, S // 128, KVL * D], BF16)  # v natural [s_p, s_t, kv*d]
        I128 = pers.tile([128, 128], BF16)
        I128f = pers.tile([128, 128], F32)
        ones1 = pers.tile([1, 128], F32R)
        maskP1 = pers.tile([128, 4, CHUNK], BF16)
        maskTd = pers.tile([128, 4, CHUNK], BF16)
        wo_pool = top.enter_context(tc.tile_pool(name="wo", bufs=1))
        wo_sb = wo_pool.tile([128, HL, HID], BF16)

        def load_consts():
            nc.sync.dma_start(I128, ap["imatb"])
            nc.sync.dma_start(I128f, ap["imat"].bitcast(F32))
            nc.sync.dma_start(ones1, ap["ones1"])
            nc.sync.dma_start(maskP1, ap["maskp1"])
            nc.sync.dma_start(maskTd, ap["masktd"])
        wf01_pool = top.enter_context(tc.tile_pool(name="wf01", bufs=1))
        p2_pool = top.enter_context(tc.tile_pool(name="pprime", bufs=2))
        o2_pool = top.enter_context(tc.tile_pool(name="uout", bufs=2))
        mt_pool = top.enter_context(tc.tile_pool(name="mt", bufs=1))
        wtx_pool = top.enter_context(tc.tile_pool(name="wtx", bufs=2))
        o_pool1 = top.enter_context(tc.tile_pool(name="osb", bufs=1))

        ch_pool = sc_ps = wtp_ps = scr_pool = xtv_pool = None
        pa = {}
        ubs_all = {}
        chst = {}
        wt2s = {}

        def load_wf(qi, pool, tag):
            """SBUF->SBUF partition-flatten of the wt2 rows (tile-tracked)."""
            nj = len(plan[qi])
            wf = pool.tile([1, nj * HL, CHUNK], F32R, tag=tag, name=f"wf{qi}")
            nc.sync.dma_start(wf, wt2s[qi].bitcast(F32R))
            return wf

        def emit_qk(sq):
            """QK projections + rope for chunk sq (width 512)."""
            ssl = slice(sq * CHUNK, (sq + 1) * CHUNK)
            xt = xt_pool.tile([128, NT, CHUNK], BF16, tag=f"xt{sq % 2}",
                              name=f"xt{sq}")
            pa[sq] = xt
            nc.sync.dma_start(xt, ap["hsT"].rearrange(
                "(t p) s -> p t s", p=128)[:, :, ssl])
            for m in range(HL + KVL):
                ps = psP.tile([128, CHUNK], F32, name="ps")
                for t in range(NT):
                    _mm(nc, ps, pa["wqk"][:, t, m * 128:(m + 1) * 128],
                        pa[sq][:, t], start=(t == 0), stop=(t == NT - 1))
                raw = raw_pool.tile([128, CHUNK], F32R, name="raw")
                nc.vector.tensor_scalar_add(raw, ps, pa["bqk"][:, m:m + 1])
                pr = psR.tile([128, CHUNK], F32, name="pr")
                _mm(nc, pr, pa["R128"], raw, start=True, stop=True)
                t1 = t_pool.tile([128, CHUNK], F32, tag="t1")
                nc.gpsimd.tensor_mul(t1, raw.bitcast(F32), pa["cosT"][:, ssl])
                t2 = t_pool.tile([128, CHUNK], F32, tag="t2")
                nc.vector.tensor_mul(t2, pr, pa["sinT"][:, ssl])
                dest = QT[:, m, ssl] if m < HL else KT[:, m - HL, ssl]
                nc.vector.tensor_add(dest, t1, t2)

        def emit_v(sq):
            """V projection for chunk sq (natural layout), bias via K=1.
            Chunks 2/3 still sit in the qk xt tiles; 0/1 reload (DMA is idle
            in this window)."""
            if sq >= 2:
                xt = pa[sq]
            else:
                xt = xtv_pool.tile([128, NT, CHUNK], BF16, tag="xtv",
                                   name=f"xtv{sq}")
                nc.sync.dma_start(xt, ap["hsT"].rearrange(
                    "(t p) s -> p t s", p=128)[:, :, sq * CHUNK:(sq + 1) * CHUNK])
            for ss in range(CHUNK // 128):
                pv = psV.tile([128, KVL * D], F32, name="pv")
                for t in range(NT):
                    _mm(nc, pv, xt[:, t, ss * 128:(ss + 1) * 128],
                        pa["wv"][:, t], start=(t == 0), stop=False)
                _mm(nc, pv, ones1, pa["bv"], start=False, stop=True)
                nc.vector.tensor_copy(V[:, sq * 4 + ss, :], pv)

        def emit_pass1(qi):
            chunks = plan[qi]
            nj = len(chunks)
            tag = f"q{qi % 2}"
            Stile = ch_pool.tile([128, nj, HL * 4], F32, tag=tag + "s")
            Emaxs = ch_pool.tile([128, nj, HL * 4], F32, tag=tag + "em")
            Eseq = ch_pool.tile([128, nj, HL * 4], F32, tag=tag + "eq")
            gt = ch_pool.tile([128, nj, HL * 4], F32, tag=tag + "g")
            PH = ch_pool.tile([128, 2, nj, HL * 4], F32, tag=tag + "ph")
            lnPH = ch_pool.tile([128, 2, nj, HL * 4], F32, tag=tag + "ln")
            Wadj = ch_pool.tile([128, nj, HL * 4], F32, tag=tag + "w")

            for t, (j, kind) in enumerate(chunks):
                ksl = slice(j * CHUNK, (j + 1) * CHUNK)
                mn = None
                if kind == "mixed":
                    mn = scr_pool.tile([128, 4, CHUNK], BF16, tag="mn")
                    nc.sync.dma_start(mn, ap["maskN"][mix_idx[(qi, j)]])
                for h in range(HL):
                    for sub in range(4):
                        col = h * 4 + sub
                        q0 = qi * CHUNK + sub * 128
                        ps = sc_ps.tile([128, CHUNK], F32, name="ps1")
                        if kind == "tri":
                            ns = (sub + 1) * 128
                            _mm(nc, ps[:, :ns], QT[:, h, q0:q0 + 128],
                                KT[:, h // 2, j * CHUNK:j * CHUNK + ns],
                                start=True, stop=False)
                            _mm(nc, ps[:, :ns], I128, maskP1[:, sub, :ns],
                                start=False, stop=True)
                        else:
                            ns = CHUNK
                            _mm(nc, ps, QT[:, h, q0:q0 + 128], KT[:, h // 2, ksl],
                                start=True, stop=(kind != "mixed"))
                            if kind == "mixed":
                                _mm(nc, ps, I128, mn[:, sub, :],
                                    start=False, stop=True)
                        scr2 = scr_pool.tile([128, CHUNK], BF16, name="scr2")
                        nc.scalar.activation(
                            scr2[:, :ns], ps[:, :ns], Act.Exp,
                            accum_out=Stile[:, t, col:col + 1])
                        nc.vector.tensor_reduce(
                            Emaxs[:, t, col:col + 1], scr2[:, :ns],
                            axis=mybir.AxisListType.X, op=Alu.max)
                if t == 0:
                    nc.vector.tensor_copy(Eseq[:, 0, :], Emaxs[:, 0, :])
                    nc.vector.tensor_copy(gt[:, 0, :], Stile[:, 0, :])
                else:
                    nc.vector.tensor_add(gt[:, t, :], Eseq[:, t - 1, :],
                                         Stile[:, t, :])
                    nc.vector.tensor_tensor(Eseq[:, t, :], Eseq[:, t - 1, :],
                                            Emaxs[:, t, :], Alu.max)

            # backward products; Hprod carries the reference's final-divide
            # factor: g_n if the globally-last kv chunk was processed, E_n if
            # not.
            last_global = any(j == NQ - 1 for (j, _) in chunks)
            extra = gt if last_global else Eseq
            nc.vector.tensor_copy(PH[:, 0, nj - 1, :], Eseq[:, nj - 1, :])
            nc.gpsimd.tensor_mul(PH[:, 1, nj - 1, :], extra[:, nj - 1, :],
                                 gt[:, nj - 1, :])
            for t in range(nj - 2, -1, -1):
                nc.gpsimd.tensor_mul(PH[:, 0, t, :], PH[:, 0, t + 1, :],
                                     Eseq[:, t, :])
                nc.gpsimd.tensor_mul(PH[:, 1, t, :], PH[:, 1, t + 1, :],
                                     gt[:, t, :])
            chst[qi] = (PH, lnPH, Wadj)

        def emit_wf(qi):
            """Ln + transpose + store of the pass-2 inject rows for qi.
            Batched after pass-1 so Exp<->Ln act-table switches stay rare."""
            nj = len(plan[qi])
            PH, lnPH, Wadj = chst[qi]
            nc.scalar.activation(lnPH, PH, Act.Ln)
            nc.vector.tensor_sub(Wadj, lnPH[:, 0], lnPH[:, 1])
            # transpose Wadj -> [nj*HL, 4, 128] (row = (t, h), col = (sub, p))
            wtp = wtp_ps.tile([nj * HL, 4, 128], F32, name="wtp")
            wadj_r = Wadj.rearrange("p n (x a) -> p n x a", a=4)
            for sub in range(4):
                nc.tensor.transpose(wtp[:, sub, :], wadj_r[:, :, :, sub], I128f)
            wt2 = wtx_pool.tile([nj * HL, CHUNK], F32, tag="wt2")
            nc.vector.tensor_copy(wt2, wtp)
            wt2s[qi] = wt2

        def emit_p2_h(qi, wf, sp_pool, up_pool):
            """pass-2 score/exp/PV h-loop for one qi into ubs tiles."""
            qsl = slice(qi * CHUNK, (qi + 1) * CHUNK)
            chunks = plan[qi]
            nj = len(chunks)
            mtload = {}
            for t, (j, kind) in enumerate(chunks):
                if kind == "mixed":
                    mt = mt_pool.tile([128, 4, CHUNK], BF16, tag=f"mt{j}",
                                      name=f"mt{j}")
                    nc.sync.dma_start(mt, ap["maskT"][mix_idx[(qi, j)]])
                    mtload[j] = mt
            ubs = []
            for h in range(HL):
                up = up_pool.tile([128, CHUNK], F32, name="up")
                for t, (j, kind) in enumerate(chunks):
                    for kp in range(2):  # kc pairs: one exp per pair
                        sp = sp_pool.tile([128, 2, CHUNK], F32, name="sp")
                        pp = p2_pool.tile([128, 2, CHUNK], BF16, name="pp")
                        for ki in range(2):
                            kc = kp * 2 + ki
                            k0 = j * CHUNK + kc * 128
                            _mm(nc, sp[:, ki, :],
                                KT[:, h // 2, k0:k0 + 128], QT[:, h, qsl],
                                start=True, stop=False)
                            if kind == "tri":
                                _mm(nc, sp[:, ki, :], I128, maskTd[:, kc, :],
                                    start=False, stop=False)
                            elif kind == "mixed":
                                _mm(nc, sp[:, ki, :], I128,
                                    mtload[j][:, kc, :],
                                    start=False, stop=False)
                            _mm(nc, sp[:, ki, :], ones1,
                                wf[:, t * HL + h, :],
                                start=False, stop=True)
                        nc.scalar.activation(pp, sp, Act.Exp)
                        for ki in range(2):
                            kc = kp * 2 + ki
                            _mm(nc, up,
                                V[:, j * 4 + kc,
                                  (h // 2) * D:(h // 2 + 1) * D],
                                pp[:, ki, :],
                                start=(t == 0 and kc == 0),
                                stop=(t == nj - 1 and kc == 3))
                ub = o2_pool.tile([128, CHUNK], BF16, tag=f"ub{h}",
                                  name=f"ub{h}")
                nc.vector.tensor_copy(ub, up)
                ubs.append(ub)
            return ubs

        def emit_oproj(qi, ubs, psO, o_pool):
            """output projection for s-chunk qi; batch 16 partials into one
            bf16 store issued from Act (zero-wait after its last copy)."""
            qsl = slice(qi * CHUNK, (qi + 1) * CHUNK)
            outr = ap["outT"].rearrange("(mh mo p) s -> p mh mo s", p=128, mh=2)
            for mh in range(2):
                obig = o_pool.tile([128, HID // 256, CHUNK], BF16, name="obig")
                for mo in range(HID // 256):
                    po = psO.tile([128, CHUNK], F32, name="po")
                    for t in range(HL):
                        _mm(nc, po,
                            wo_sb[:, t, (mh * 8 + mo) * 128:(mh * 8 + mo + 1) * 128],
                            ubs[t], start=(t == 0), stop=(t == HL - 1))
                    nc.vector.tensor_copy(obig[:, mo, :], po)
                nc.sync.dma_start(outr[:, mh, :, qsl], obig)

        # ------- phase A + pass 1 + early pass 2, interleaved -------
        with ExitStack() as ph1:
            ch_pool = ph1.enter_context(tc.tile_pool(name="chain", bufs=1))
            sc_ps = ph1.enter_context(tc.tile_pool(name="scps", bufs=2, space="PSUM"))
            wtp_ps = ph1.enter_context(tc.tile_pool(name="wtpps", bufs=1, space="PSUM"))
            scr_pool = ph1.enter_context(tc.tile_pool(name="scr", bufs=2))
            raw_pool = ph1.enter_context(tc.tile_pool(name="raw", bufs=2))
            t_pool = ph1.enter_context(tc.tile_pool(name="ropetmp", bufs=1))
            w_pool = ph1.enter_context(tc.tile_pool(name="wsb", bufs=1))

            wv_sb = w_pool.tile([128, NT, KVL * D], BF16)
            pa["wv"] = wv_sb
            bqk = w_pool.tile([128, HL + KVL], F32)
            nc.sync.dma_start(bqk, ap["bqk"])
            pa["bqk"] = bqk
            bv = w_pool.tile([1, KVL * D], F32R)
            nc.sync.dma_start(bv, ap["bv"])
            pa["bv"] = bv
            R128 = w_pool.tile([128, 128], F32R)
            nc.sync.dma_start(R128, ap["rmat"])
            pa["R128"] = R128
            xt_pool = ph1.enter_context(tc.tile_pool(name="xt", bufs=1))
            with ExitStack() as psa:
                wqkcs_pool = psa.enter_context(tc.tile_pool(name="wqkcs", bufs=1))
                wqk_sb = wqkcs_pool.tile([128, NT, (HL + KVL) * D], BF16)
                wqk_r = ap["wqk"].rearrange("(t p) m -> p t m", p=128)
                nc.sync.dma_start(wqk_sb[:, :, :384], wqk_r[:, :, :384])
                nc.sync.dma_start(wqk_sb[:, :, 384:], wqk_r[:, :, 384:])
                pa["wqk"] = wqk_sb
                cosT = wqkcs_pool.tile([128, S], BF16, tag="cos")
                nc.sync.dma_start(cosT, ap["cosT"])
                pa["cosT"] = cosT
                sinT = wqkcs_pool.tile([128, S], BF16, tag="sin")
                nc.sync.dma_start(sinT, ap["sinT"])
                pa["sinT"] = sinT
                psP = psa.enter_context(
                    tc.tile_pool(name="psP", bufs=2, space="PSUM"))
                psR = psa.enter_context(
                    tc.tile_pool(name="psR", bufs=2, space="PSUM"))
                emit_qk(0)
                load_consts()
                emit_qk(1)
                nc.sync.dma_start(
                    wo_sb, ap["wo"].rearrange("(t p) m -> p t m", p=128))
                emit_pass1(0)
                emit_qk(2)
                nc.sync.dma_start(
                    pa["wv"], ap["wv"].rearrange("(t p) m -> p t m", p=128))
                emit_pass1(1)
                emit_qk(3)
                emit_wf(0)
                emit_wf(1)
            with ExitStack() as psb:
                xtv_pool = psb.enter_context(tc.tile_pool(name="xtv", bufs=2))
                psV = psb.enter_context(
                    tc.tile_pool(name="psV", bufs=1, space="PSUM"))
                sp0 = psb.enter_context(
                    tc.tile_pool(name="sp0", bufs=1, space="PSUM"))
                u0 = psb.enter_context(
                    tc.tile_pool(name="u0", bufs=1, space="PSUM"))
                psOb = psb.enter_context(
                    tc.tile_pool(name="psOb", bufs=1, space="PSUM"))
                emit_pass1(2)
                emit_v(0)
                emit_v(2)
                wf0 = load_wf(0, wf01_pool, "wf01")
                ubs_all[0] = emit_p2_h(0, wf0, sp0, u0)
                emit_pass1(3)
                emit_v(1)
                emit_v(3)
                emit_wf(2)
                emit_oproj(0, ubs_all[0], psOb, o_pool1)
                wf1 = load_wf(1, wf01_pool, "wf01")
                ubs_all[1] = emit_p2_h(1, wf1, sp0, u0)
                emit_wf(3)
                emit_oproj(1, ubs_all[1], psOb, o_pool1)

        # ---------------- rest of pass 2 + output projections ----------------
        with ExitStack() as ph2:
            wf23_pool = ph2.enter_context(tc.tile_pool(name="wf23", bufs=2))
            s2_ps = ph2.enter_context(tc.tile_pool(name="s2ps", bufs=2, space="PSUM"))
            u_ps = ph2.enter_context(tc.tile_pool(name="ups", bufs=1, space="PSUM"))
            psO = ph2.enter_context(tc.tile_pool(name="psO", bufs=2, space="PSUM"))

            wf2 = load_wf(2, wf23_pool, "wf23")
            ubs_all[2] = emit_p2_h(2, wf2, s2_ps, u_ps)
            wf3 = load_wf(3, wf23_pool, "wf23")
            emit_oproj(2, ubs_all[2], psO, o_pool1)
            ubs_all[3] = emit_p2_h(3, wf3, s2_ps, u_ps)
            emit_oproj(3, ubs_all[3], psO, o_pool1)


def _build_program(plan, mask_blocks):
    nc = bacc.Bacc("TRN2", target_bir_lowering=False, debug=False,
                   enable_asserts=False, num_devices=NCORES)
    ap = {}
    ap["hsT"] = nc.dram_tensor("hsT", [HID, S], BF16, kind="ExternalInput").ap()
    ap["wqk"] = nc.dram_tensor("wqk", [HID, (HL + KVL) * D], BF16, kind="ExternalInput").ap()
    ap["wv"] = nc.dram_tensor("wv", [HID, KVL * D], BF16, kind="ExternalInput").ap()
    ap["wo"] = nc.dram_tensor("wo", [HL * D, HID], BF16, kind="ExternalInput").ap()
    ap["bqk"] = nc.dram_tensor("bqk", [D, HL + KVL], F32, kind="ExternalInput").ap()
    ap["bv"] = nc.dram_tensor("bv", [1, KVL * D], F32R, kind="ExternalInput").ap()
    ap["cosT"] = nc.dram_tensor("cosT", [D, S], BF16, kind="ExternalInput").ap()
    ap["sinT"] = nc.dram_tensor("sinT", [D, S], BF16, kind="ExternalInput").ap()
    ap["rmat"] = nc.dram_tensor("rmat", [D, D], F32R, kind="ExternalInput").ap()
    ap["imat"] = nc.dram_tensor("imat", [128, 128], F32R, kind="ExternalInput").ap()
    ap["imatb"] = nc.dram_tensor("imatb", [128, 128], BF16, kind="ExternalInput").ap()
    ap["ones1"] = nc.dram_tensor("ones1", [1, 128], F32R, kind="ExternalInput").ap()
    ap["maskp1"] = nc.dram_tensor("maskp1", [128, 4, CHUNK], BF16, kind="ExternalInput").ap()
    ap["masktd"] = nc.dram_tensor("masktd", [128, 4, CHUNK], BF16, kind="ExternalInput").ap()
    nmix = max(1, len(mask_blocks))
    ap["maskN"] = nc.dram_tensor("maskN", [nmix, 128, 4, CHUNK], BF16, kind="ExternalInput").ap()
    ap["maskT"] = nc.dram_tensor("maskT", [nmix, 128, 4, CHUNK], BF16, kind="ExternalInput").ap()
    ap["wscr"] = nc.dram_tensor("wscr", [NQ, NQ * HL, CHUNK], F32, kind="ExternalOutput").ap()
    ap["outT"] = nc.dram_tensor("outT", [HID, S], BF16, kind="ExternalOutput").ap()
    mix_idx = {qj: i for i, qj in enumerate(mask_blocks)}

    with tile.TileContext(nc) as tc:
        _emit(tc, ap, plan, mix_idx)
    nc.compile()
    return nc


def _host_inputs(inputs, mask_blocks):
    hs = np.asarray(inputs["hidden_states"], dtype=np.float32)
    am = np.asarray(inputs["attention_mask"], dtype=np.float32)
    Wq = np.asarray(inputs["Wq"], dtype=np.float32)
    bq = np.asarray(inputs["bq"], dtype=np.float32)
    Wk = np.asarray(inputs["Wk"], dtype=np.float32)
    bk = np.asarray(inputs["bk"], dtype=np.float32)
    Wv = np.asarray(inputs["Wv"], dtype=np.float32)
    bv_ = np.asarray(inputs["bv"], dtype=np.float32)
    Wo = np.asarray(inputs["Wo"], dtype=np.float32)

    cosT, sinT = _rope_tables()
    R = np.zeros((D, D), dtype=np.float32)
    R[64 + np.arange(64), np.arange(64)] = -1.0   # out[d'<64] = -q[d'+64]
    R[np.arange(64), 64 + np.arange(64)] = 1.0    # out[d'>=64] = q[d'-64]
    I = np.eye(128, dtype=np.float32)

    mp1 = np.zeros((128, 4, CHUNK), dtype=np.float32)
    for sub in range(4):
        qpos = sub * 128 + np.arange(128)
        mp1[:, sub, :] = np.where(
            np.arange(CHUNK)[None, :] <= qpos[:, None], 0.0, -1e9)
    mtd = np.zeros((128, 4, CHUNK), dtype=np.float32)
    for kc in range(4):
        kpos = kc * 128 + np.arange(128)
        mtd[:, kc, :] = np.where(
            kpos[:, None] <= np.arange(CHUNK)[None, :], 0.0, -1e9)

    Wq4 = (Wq * SCALE).reshape(HID, H, D)
    bq4 = (bq * SCALE).reshape(H, D)
    Wk4 = Wk.reshape(HID, HKV, D)
    bk4 = bk.reshape(HKV, D)
    Wv4 = Wv.reshape(HID, HKV, D)
    bv4 = bv_.reshape(HKV, D)
    Wo4 = Wo.reshape(H, D, HID)

    nmix = max(1, len(mask_blocks))
    in_maps = []
    for c in range(NCORES):
        b, hg = divmod(c, NCORES // B)
        qh = slice(hg * HL, (hg + 1) * HL)
        kvh = slice(hg * KVL, (hg + 1) * KVL)
        wqk = np.concatenate([
            Wq4[:, qh].reshape(HID, HL * D),
            Wk4[:, kvh].reshape(HID, KVL * D)], axis=1)
        bqk = np.concatenate([bq4[qh], bk4[kvh]], axis=0).T  # [D, HL+KVL]
        mN = np.zeros((nmix, 128, 4, CHUNK), dtype=np.float32)
        mT = np.zeros((nmix, 128, 4, CHUNK), dtype=np.float32)
        for i, (qi, j) in enumerate(mask_blocks):
            blk = am[b, 0, qi * CHUNK:(qi + 1) * CHUNK, j * CHUNK:(j + 1) * CHUNK]
            mN[i] = blk.reshape(4, 128, CHUNK).transpose(1, 0, 2)
            mT[i] = blk.T.reshape(4, 128, CHUNK).transpose(1, 0, 2)
        in_maps.append({
            "hsT": _bf16(hs[b].T),
            "wqk": _bf16(wqk),
            "wv": _bf16(Wv4[:, kvh].reshape(HID, KVL * D)),
            "wo": _bf16(Wo4[qh].reshape(HL * D, HID)),
            "bqk": np.ascontiguousarray(bqk),
            "bv": _f32r_round(bv4[kvh].reshape(1, KVL * D)),
            "cosT": _bf16(cosT),
            "sinT": _bf16(sinT),
            "rmat": R,
            "imat": I,
            "imatb": _bf16(I),
            "ones1": np.ones((1, 128), dtype=np.float32),
            "maskp1": _bf16(mp1),
            "masktd": _bf16(mtd),
            "maskN": _bf16(mN),
            "maskT": _bf16(mT),
        })
    return in_maps


def get_program(inputs):
    am = np.asarray(inputs["attention_mask"], dtype=np.float32)
    plan, mask_blocks = _classify_mask(am)
    key = (str(plan), str(mask_blocks))
    if key not in _CACHE:
        _CACHE[key] = _build_program(plan, mask_blocks)
    return _CACHE[key], plan, mask_blocks


def run(inputs, **spmd_kwargs):
    nc, plan, mask_blocks = get_program(inputs)
    in_maps = _host_inputs(inputs, mask_blocks)
    res = run_bass_kernel_spmd(nc, in_maps, core_ids=list(range(NCORES)),
                               **spmd_kwargs)
    bo = np.asarray(inputs["bo"], dtype=np.float32)
    out = np.empty((B, S, HID), dtype=np.float32)
    gpb = NCORES // B
    for b in range(B):
        acc = np.zeros((HID, S), dtype=np.float32)
        for c in range(b * gpb, (b + 1) * gpb):
            acc += np.asarray(res.results[c]["outT"], dtype=np.float32)
        out[b] = acc.T + bo
    return out, res


def kernel(**inputs) -> np.ndarray:
    out, _ = run(inputs)
    return out
